# revision 35
# baseline (speedup 1.0000x reference)
"""Navier-Stokes PINN kernel for 8x Trainium2 NeuronCores.

Math: the reference MLP uses ReLU activations, so the network is piecewise
linear in its inputs; all second derivatives vanish and the PDE residuals
collapse to
    u = dpsi/dy,  v = -dpsi/dx,  p = MLP(z)[1],  f = dp/dx,  g = dp/dy.
Everything comes from one forward pass plus two forward-mode tangent streams
(d/dx, d/dy) through the masked linear layers, with the 5 outputs produced by
one accumulated matmul against a host-assembled Wfin.

Numerics: matmuls run in float32r (single-pass, ~13 mantissa bits, 4x faster
than fp32 on the PE). The ReLU masks must match the fp32 reference masks to
~2^-15 relative accuracy, which a single f32r pass cannot deliver, so each
forward weight/activation pair uses a 3-product compensated scheme
    W^T H ~= Whi~Hhi + Wres~Hhi + Whi~Hres   (hi = f32r round, res = residual)
leaving ~2^-26 error; an all-2-product scheme cannot pass the 2e-2 gate (the
uncorrected source alone costs ~2.3e-2), but the LAST hidden layer alone runs
2-product (measured cost 7.5e-3 of the budget, total rel err 7.6e-3). Tangent
streams and the output projection are plain single-pass f32r. The input layer
(K=3) folds all three products into ONE K=9 matmul by stacking
[Whi; Whi; Wres] against [zhi; zres; zhi] at 32-aligned partitions.

Custom DVE ops (single-uop, registered per-NEFF):
    NS_GATE_POS  G = T where A>0 else 0  - tangent gating with no mask tile
    NS_RELU_RES  Hres = A-Hhi where A>0  - f32r residual straight from PSUM

Schedule: two-stage software pipeline. The A-stream (forward matmuls +
rounded-relu pairs) runs two stages ahead of the T-stream (tangent matmuls,
gating, output projection), interleaved across block boundaries:
    [A1 A2] [A3 T0] [A4 T1] [A5 T2] [A1' T3] [A2' OUT] [A3' T0'] ...
so every inter-engine hop (Act relu-round -> DVE residual/gating) has a full
~4us stage of slack and the PE stays ~95% busy. Engines: Act = rounded relu
(one op per A half, straight from PSUM) + out bias; DVE = gating + residuals
(only PSUM-capable engine besides Act); Pool = input-mask, tangent init,
z residuals. PSUM: A ring 3x1 bank, T0/T1 rings 1x2 banks, out 1 = 8 banks.

Per-block (512 points) PE floor: 2 input + 44 forward + 32 tangent + 6 output
products x 512 rows = 17.9us; 16 blocks/core ~= 287us + ~20us pipeline
overhead (cost-model timeline 307us, x1.285 HW calibration ~= 394us).

Sharding: pure data parallel, 8192 points per core, weights replicated.
"""

import numpy as np

NCORES = 8
N_TOTAL = 65536
NPC = N_TOTAL // NCORES  # points per core
HID = 256
NL = 4  # hidden->hidden layers (L=6 total: in + 4 hidden + out)
B = 512  # points per block
NB = NPC // B
P = 128
NH = HID // P  # feature halves

_NC_CACHE = {}
MM_LABELS = {}


def _register_dve_ops():
    """Two fused single-uop custom DVE ops (registered per-NEFF):

    NS_GATE_POS  out = in0 where in1 > 0 else 0   (tangent gating, no mask)
    NS_RELU_RES  out = in0-in1 where in0 > 0 else 0 (f32r residual from PSUM)
    """
    import numpy as np

    from concourse import dve_ops as D
    from concourse.dve_spec import Spec, Src0, Src1, Zero, select

    if any(op.name == "NS_GATE_POS" for op in D.OPS):
        return
    from concourse.dve_spec import lower
    from concourse.dve_uop import DveOpSpec

    defs = [
        (
            "NS_GATE_POS",
            Spec(
                body=select(Zero < Src1, Src0, Zero),
                reference=lambda in0, in1, s0, s1, imm2: np.where(
                    in1 > 0, in0, 0.0
                ).astype(np.float32),
            ),
        ),
        (
            "NS_RELU_RES",
            Spec(
                body=select(Zero < Src0, Src0 - Src1, Zero),
                reference=lambda in0, in1, s0, s1, imm2: np.where(
                    in0 > 0, in0 - in1, 0.0
                ).astype(np.float32),
            ),
        ),
    ]
    for name, spec in defs:
        opcode = D._CUSTOM_DVE_ROW_BASE + len(D._SUB_OPCODE_FOR_NAME)
        # pin the sha to whatever this environment's lowering produces
        # (validated bit-exact on device for both ops, incl. PSUM inputs,
        # broadcast predicates and f32r outputs)
        shas = {}
        for ver in ("v3", "v4"):
            try:
                s = DveOpSpec(
                    name=name,
                    opcode=opcode,
                    uops=lower(spec, ver=ver),
                    rd1_en=True,
                ).sha(ver)
            except Exception:
                continue
            shas[ver] = s
        op = D.DveOp(name, spec, subdim=False, uops_sha=shas)
        D.OPS.append(op)
        D._SUB_OPCODE_FOR_NAME[name] = opcode
    return


def _build(zero_bias: bool):
    import concourse.tile as tile
    from concourse import bacc, mybir
    from concourse import dve_ops as D

    _register_dve_ops()
    gate_op = next(op for op in D.OPS if op.name == "NS_GATE_POS")
    res_op = next(op for op in D.OPS if op.name == "NS_RELU_RES")

    f32 = mybir.dt.float32
    f32r = mybir.dt.float32r
    Relu = mybir.ActivationFunctionType.Relu
    Ident = mybir.ActivationFunctionType.Identity
    Copy = mybir.ActivationFunctionType.Copy
    mult = mybir.AluOpType.mult
    sub = mybir.AluOpType.subtract
    is_gt = mybir.AluOpType.is_gt

    nc = bacc.Bacc(
        "TRN2",
        target_bir_lowering=False,
        debug=False,
        enable_asserts=False,
        num_devices=NCORES,
    )

    zt_d = nc.dram_tensor("zt", (3, NPC), f32, kind="ExternalInput").ap()
    win_d = nc.dram_tensor("win", (3, HID), f32, kind="ExternalInput").ap()
    wint_d = nc.dram_tensor("wint", (HID, 2), f32, kind="ExternalInput").ap()
    bin_d = nc.dram_tensor("bin", (HID, 1), f32, kind="ExternalInput").ap()
    wh_d = nc.dram_tensor("wh", (NL, HID, HID), f32, kind="ExternalInput").ap()
    bh_d = nc.dram_tensor("bh", (NL, HID, 1), f32, kind="ExternalInput").ap()
    wfin_d = nc.dram_tensor("wfin", (3 * HID, 5), f32, kind="ExternalInput").ap()
    bfin_d = nc.dram_tensor("bfin", (5, 1), f32, kind="ExternalInput").ap()
    out_d = nc.dram_tensor("out", (5, NPC), f32, kind="ExternalOutput").ap()

    with tile.TileContext(nc) as tc:
        with (
            tc.tile_pool(name="weights", bufs=1) as wpool,
            tc.tile_pool(name="zin", bufs=3) as zpool,
            tc.tile_pool(name="acts", bufs=3) as hpool,
            tc.tile_pool(name="tans", bufs=2) as gpool,
            tc.tile_pool(name="masks", bufs=2) as mpool,
            tc.tile_pool(name="outs", bufs=2) as opool,
            tc.tile_pool(name="ps", bufs=1, space="PSUM") as ps,
        ):
            # ---- z prep for blocks 0/1 first: their DMAs and rounding ops
            # sit ahead of the weight-staging queue traffic, so block 0's
            # input matmul can start as soon as the input weights land ----
            warm = zpool.tile([1, 2], f32, tag="warm", name="warm")
            nc.vector.memset(warm[:], 0.0)
            nc.scalar.activation(warm[:], warm[:], Relu)
            # burn through the PE p-state ramp on zeros while the first z and
            # weight DMAs are in flight
            wml = zpool.tile([P, P], f32, tag="wml", name="wml")
            nc.vector.memset(wml[:], 0.0)
            _NWARM = 8
            wmr = zpool.tile([P, B], f32, tag="wmr", name="wmr")
            nc.gpsimd.memset(wmr[:], 0.0)
            wmp = ps.tile([P, 2 * B], f32, tag="T0", name="T", bufs=1)
            for wi in range(_NWARM):
                nc.tensor.matmul(
                    wmp[:, (wi % 2) * B : (wi % 2) * B + 64],
                    wml[:].bitcast(f32r),
                    wmr[:, 0:64].bitcast(f32r),
                    start=True,
                    stop=True,
                )

            zprep = {}

            def prep_z(ib):
                zt = zpool.tile([3, B], f32, tag="zt", name="zt")
                nc.sync.dma_start(zt[:], zt_d[:, ib * B : (ib + 1) * B])
                zc = zpool.tile([96, B], f32, tag="zc", name="zc")
                if ib < 3:
                    # first touch of each ring slot: clear the gap partitions
                    # (0 * garbage could be NaN otherwise)
                    nc.gpsimd.memset(zc[:], 0.0)
                nc.scalar.activation(zc[0:3, :].bitcast(f32r), zt[:], Copy)
                nc.scalar.activation(zc[64:67, :].bitcast(f32r), zt[:], Copy)
                nc.gpsimd.tensor_tensor(
                    zc[32:35, :].bitcast(f32r), zt[:], zc[0:3, :], sub
                )
                zprep[ib] = zc

            prep_z(0)
            prep_z(1)

            # ---- one-time weight staging (hi = f32r rounding, res = W - hi).
            # Rounds for later-used weights go on DVE and their residuals on
            # Pool, so block 0's relu/gating ops aren't queued behind the
            # whole staging burst on one engine. ----
            def stage_pair(name, shape, src_ap, late=False):
                t = wpool.tile(shape, f32, tag=name, name=name)
                nc.sync.dma_start(t[:], src_ap)
                hi = wpool.tile(shape, f32, tag=name + "h", name=name + "h")
                if late:
                    nc.vector.tensor_scalar(
                        hi[:].bitcast(f32r), t[:], 1.0, None, mult
                    )
                else:
                    nc.scalar.activation(hi[:].bitcast(f32r), t[:], Copy)
                rs = wpool.tile(shape, f32, tag=name + "s", name=name + "s")
                eng = nc.gpsimd if late else nc.vector
                eng.tensor_tensor(rs[:].bitcast(f32r), t[:], hi[:], sub)
                return hi, rs

            # stacked input weight [Whi; Wres; Whi]: the K=3 input layer's
            # three compensated products collapse into one K=9 matmul
            # against [zhi; zres; zhi].
            winf = wpool.tile([3, HID], f32, tag="winf", name="winf")
            nc.sync.dma_start(winf[:], win_d[:, :])
            winc = wpool.tile([96, HID], f32, tag="winc", name="winc")
            nc.gpsimd.memset(winc[:], 0.0)
            nc.vector.tensor_scalar(
                winc[0:3, :].bitcast(f32r), winf[:], 1.0, None, mult
            )
            nc.vector.tensor_scalar(
                winc[32:35, :].bitcast(f32r), winf[:], 1.0, None, mult
            )
            nc.gpsimd.tensor_tensor(
                winc[64:67, :].bitcast(f32r), winf[:], winc[0:3, :], sub
            )
            wint_t = []
            bin_t = []
            for h in range(NH):
                w = wpool.tile([P, 2], f32, tag=f"wint{h}", name=f"wint{h}")
                nc.sync.dma_start(w[:], wint_d[h * P : (h + 1) * P, :])
                wint_t.append(w)
                if not zero_bias:
                    b = wpool.tile([P, 1], f32, tag=f"bin{h}", name=f"bin{h}")
                    nc.sync.dma_start(b[:], bin_d[h * P : (h + 1) * P, :])
                    bin_t.append(b)
            wh_hi = {}
            wh_rs = {}
            bh_t = {}
            for li in range(NL):
                for k in range(NH):
                    for h in range(NH):
                        wh_hi[li, k, h], wh_rs[li, k, h] = stage_pair(
                            f"wh{li}{k}{h}",
                            [P, P],
                            wh_d[li, k * P : (k + 1) * P, h * P : (h + 1) * P],
                            late=(li >= 2),
                        )
                if not zero_bias:
                    for h in range(NH):
                        b = wpool.tile([P, 1], f32, tag=f"bh{li}{h}", name=f"bh{li}{h}")
                        nc.sync.dma_start(b[:], bh_d[li, h * P : (h + 1) * P, :])
                        bh_t[li, h] = b
            wfin_t = []
            for k in range(3 * NH):
                w = wpool.tile([P, 5], f32, tag=f"wfin{k}", name=f"wfin{k}")
                nc.sync.dma_start(w[:], wfin_d[k * P : (k + 1) * P, :])
                wr = wpool.tile([P, 5], f32, tag=f"wfin{k}r", name=f"wfin{k}r")
                nc.vector.tensor_scalar(wr[:].bitcast(f32r), w[:], 1.0, None, mult)
                wfin_t.append(wr)
            bfin_t = wpool.tile([5, 1], f32, tag="bfin", name="bfin")
            nc.sync.dma_start(bfin_t[:], bfin_d[:, :])

            # ---- per-block pipeline ----
            # Two-stage software pipeline: the A-stream (forward matmuls +
            # relu pairs) runs two stages ahead of the T-stream (tangent
            # matmuls + gating + output), interleaved across block
            # boundaries:
            #   [A1 A2] [A3 T0] [A4 T1] [A5 T2] [A1' T3] [A2' OUT] [A3' T0'] ...
            # Every inter-engine chain (A-mm -> relu -> residual/gating ->
            # dependent matmul) then has a full stage (~4 us) of slack, so
            # the PE never waits on Act/DVE/Pool latency.
            hhi_of = {}  # (block, layer, half) -> rounded relu tile
            hrs_of = {}  # (block, layer, half) -> residual tile
            g_of = {}  # block -> [G tile per half]

            def bias_ap(li, h):
                bt = bin_t if li == 0 else [bh_t[li - 1, 0], bh_t[li - 1, 1]]
                return bt[h][:, 0:1]

            def emit_pair(a_half, b, li, h, last, no_res=False):
                """Hhi (+ H', Hres) + bookkeeping for one A half-tile."""
                hhi = hpool.tile([P, B], f32, tag="Hh", name="Hh", bufs=8)
                if zero_bias:
                    nc.scalar.activation(hhi[:].bitcast(f32r), a_half[:], Relu)
                else:
                    nc.scalar.activation(
                        hhi[:].bitcast(f32r), a_half[:], Relu, bias=bias_ap(li, h)
                    )
                hhi_of[b, li, h] = hhi
                if last or no_res:
                    return
                hrs = hpool.tile([P, B], f32, tag="Hs", name="Hs", bufs=8)
                if zero_bias:
                    nc.vector._custom_dve(
                        res_op, out=hrs[:].bitcast(f32r), in0=a_half[:], in1=hhi[:]
                    )
                else:
                    ht = hpool.tile([P, B], f32, tag="H", name="H", bufs=6)
                    nc.scalar.activation(ht[:], a_half[:], Relu, bias=bias_ap(li, h))
                    nc.vector.tensor_tensor(hrs[:].bitcast(f32r), ht[:], hhi[:], sub)
                hrs_of[b, li, h] = hrs

            def a_stage(b, i):
                """Forward stage i of block b: A matmuls + relu pair."""
                if i == 0:
                    zc = zprep.pop(b)
                    for h in range(NH):
                        a = ps.tile([P, B], f32, tag="A", name="A", bufs=3)
                        mi = nc.tensor.matmul(
                            a[:],
                            winc[:].bitcast(f32r)[:, h * P : (h + 1) * P],
                            zc[:].bitcast(f32r),
                            start=True,
                            stop=True,
                        )
                        MM_LABELS[mi.ins.name] = f"A1.h{h}" 
                        emit_pair(a, b, 0, h, last=False)
                    # input tangent init: G1 = M1 * Win-row ([x|y] per half)
                    mt = []
                    for h in range(NH):
                        m = mpool.tile([P, B], f32, tag="M", name="M", bufs=4)
                        nc.gpsimd.tensor_scalar(
                            m[:], hhi_of[b, 0, h][:], 0.0, None, is_gt
                        )
                        mt.append(m)
                    Gs = []
                    for h in range(NH):
                        gt = gpool.tile([P, 2 * B], f32, tag=f"G{h}", name=f"G{h}")
                        for d in range(2):
                            nc.gpsimd.tensor_scalar(
                                gt[:, d * B : (d + 1) * B].bitcast(f32r),
                                mt[h][:],
                                wint_t[h][:, d : d + 1],
                                None,
                                mult,
                            )
                        Gs.append(gt)
                    g_of[b] = Gs
                    return
                li = i - 1  # weight layer index
                # Last hidden layer: 2-product compensation (drop Whi*Hres).
                # Its mask noise costs a measured 7.5e-3 of the 2e-2 error
                # budget (total 7.6e-3, >2.6x margin) and saves 4 PE products
                # per block plus the layer-3 residuals. Extending to layer 3
                # as well measured 1.20e-2 total (worst output 1.74e-2) -
                # too thin a margin to ship.
                two_pass = i == 4
                for h in range(NH):
                    a = ps.tile([P, B], f32, tag="A", name="A", bufs=3)
                    prods = []
                    for k in range(NH):
                        whi = wh_hi[li, k, h][:].bitcast(f32r)
                        wrs = wh_rs[li, k, h][:].bitcast(f32r)
                        hh = hhi_of[b, i - 1, k][:].bitcast(f32r)
                        prods += [(whi, hh), (wrs, hh)]
                    if not two_pass:
                        for k in range(NH):
                            whi = wh_hi[li, k, h][:].bitcast(f32r)
                            hs = hrs_of[b, i - 1, k][:].bitcast(f32r)
                            prods.append((whi, hs))
                    for ip, (lhs, rhs) in enumerate(prods):
                        mi = nc.tensor.matmul(
                            a[:],
                            lhs,
                            rhs,
                            start=(ip == 0),
                            stop=(ip == len(prods) - 1),
                        )
                        MM_LABELS[mi.ins.name] = f"A{i + 1}.h{h}.p{ip}" 
                    emit_pair(a, b, i, h, last=(i == 4), no_res=(i == 3))

            def t_stage(b, j):
                """Tangent stage j (j<4: T matmuls + gating; j=4: output)."""
                if j < 4:
                    Gs = g_of[b]
                    nGs = []
                    for h in range(NH):
                        tp = ps.tile(
                            [P, 2 * B], f32, tag=f"T{h}", name="T", bufs=1
                        )
                        for d in range(2):
                            for k in range(NH):
                                mi = nc.tensor.matmul(
                                    tp[:, d * B : (d + 1) * B],
                                    wh_hi[j, k, h][:].bitcast(f32r),
                                    Gs[k][:, d * B : (d + 1) * B].bitcast(f32r),
                                    start=(k == 0),
                                    stop=(k == NH - 1),
                                )
                                MM_LABELS[mi.ins.name] = f"T{j}.h{h}.d{d}.k{k}" 
                        gt = gpool.tile([P, 2 * B], f32, tag=f"G{h}", name=f"G{h}")
                        pred = (
                            hhi_of[b, j + 1, h][:]
                            .unsqueeze(1)
                            .broadcast_to((P, 2, B))
                        )
                        nc.vector._custom_dve(
                            gate_op,
                            out=gt[:].bitcast(f32r).rearrange(
                                "p (d b) -> p d b", d=2
                            ),
                            in0=tp[:].rearrange("p (d b) -> p d b", d=2),
                            in1=pred,
                        )
                        nGs.append(gt)
                    g_of[b] = nGs
                    return
                # output projection
                Gs = g_of.pop(b)
                ops = ps.tile([5, B], f32, tag="O", name="O", bufs=1)
                chunks = [
                    (0, hhi_of[b, 4, 0][:].bitcast(f32r)),
                    (1, hhi_of[b, 4, 1][:].bitcast(f32r)),
                ]
                # wfin chunk index for (d, h) is 2 + d*NH + h; order chunks so
                # each gating half (h) unblocks its two dirs back to back
                for h in range(NH):
                    for d in range(2):
                        chunks.append(
                            (2 + d * NH + h, Gs[h][:, d * B : (d + 1) * B].bitcast(f32r))
                        )
                for i_c, (k, rhs) in enumerate(chunks):
                    mi = nc.tensor.matmul(
                        ops[:],
                        wfin_t[k][:].bitcast(f32r),
                        rhs,
                        start=(i_c == 0),
                        stop=(i_c == 5),
                    )
                    MM_LABELS[mi.ins.name] = f"OUT.c{k}" 
                osb = opool.tile([5, B], f32, tag="osb", name="osb")
                nc.scalar.activation(osb[:], ops[:], Ident, bias=bfin_t[:, 0:1])
                nc.sync.dma_start(out_d[:, b * B : (b + 1) * B], osb[:])
                for key in [k for k in hhi_of if k[0] == b]:
                    del hhi_of[key]
                for key in [k for k in hrs_of if k[0] == b]:
                    del hrs_of[key]

            total = NB * 5
            for s in range(total + 2):
                if s < total:
                    b, i = divmod(s, 5)
                    if i == 0 and b + 2 < NB:
                        prep_z(b + 2)
                    a_stage(b, i)
                if s >= 2:
                    b2, j = divmod(s - 2, 5)
                    t_stage(b2, j)

    nc.compile()
    return nc


def _get_nc(zero_bias=True):
    if zero_bias not in _NC_CACHE:
        _NC_CACHE[zero_bias] = _build(zero_bias)
    return _NC_CACHE[zero_bias]


def kernel(x, y, t, Win, b_in, Wh, b_h, Wout, b_out, _trace=False):
    from concourse import bass_utils

    x = np.asarray(x, np.float32)
    y = np.asarray(y, np.float32)
    t = np.asarray(t, np.float32)
    Win = np.asarray(Win, np.float32)
    b_in = np.asarray(b_in, np.float32)
    Wh = np.asarray(Wh, np.float32)
    b_h = np.asarray(b_h, np.float32)
    Wout = np.asarray(Wout, np.float32)
    b_out = np.asarray(b_out, np.float32)

    z = np.ascontiguousarray(
        np.stack([x[:, 0], y[:, 0], t[:, 0]], axis=0)
    )  # (3, N)
    wint = np.ascontiguousarray(Win[0:2, :].T)  # (HID, 2)
    binc = np.ascontiguousarray(b_in.reshape(HID, 1))
    bhc = np.ascontiguousarray(b_h.reshape(NL, HID, 1))
    wfin = np.zeros((3 * HID, 5), np.float32)
    wfin[2 * HID : 3 * HID, 0] = Wout[:, 0]  # u = dpsi/dy
    wfin[HID : 2 * HID, 1] = -Wout[:, 0]  # v = -dpsi/dx
    wfin[0:HID, 2] = Wout[:, 1]  # p
    wfin[HID : 2 * HID, 3] = Wout[:, 1]  # f = dp/dx
    wfin[2 * HID : 3 * HID, 4] = Wout[:, 1]  # g = dp/dy
    bfin = np.zeros((5, 1), np.float32)
    bfin[2, 0] = b_out[1]

    zero_bias = not (np.any(b_in) or np.any(b_h))
    nc = _get_nc(zero_bias)
    in_maps = []
    for c in range(NCORES):
        in_maps.append(
            {
                "zt": np.ascontiguousarray(z[:, c * NPC : (c + 1) * NPC]),
                "win": Win,
                "wint": wint,
                "bin": binc,
                "wh": Wh,
                "bh": bhc,
                "wfin": wfin,
                "bfin": bfin,
            }
        )
    res = bass_utils.run_bass_kernel_spmd(
        nc, in_maps, core_ids=list(range(NCORES)), trace=_trace
    )
    kernel._last_results = res
    full = np.concatenate(
        [res.results[c]["out"] for c in range(NCORES)], axis=1
    )  # (5, N)
    return np.ascontiguousarray(full[:, :, None].astype(np.float32))
